# revision 1
# baseline (speedup 1.0000x reference)
"""NeuralCDE RK4 solver as a Bass/Tile kernel on 8 Trainium2 cores.

Data-parallel over batch: B=1024 -> 128 rows per core (one partition tile).
The 127-step RK4 scan is fully unrolled; per stage:
    mm1 (PE)  : h_psum[128m,128b] = W1z.T @ zT_stage
    relu (ACT): hS = relu(h_psum + bias1(t))     (time channel folded in bias)
    mm2 (PE)  : f_psum[128b,512]  = ones.T@b2 + hS.T @ W2   (accumulated)
    tanh (ACT): fS = tanh(f_psum)
    mul  (DVE): u = fS * g(step,stage)           (g broadcast along h via AP)
    red  (DVE): k_nat[128b,64] = sum_c u
    T    (PE) : k^T accumulated into acc_psum    (RK4 weights pre-folded in g)
    stt  (DVE): z_stage_next = k^T * alpha + zT
State z^T lives in one big SBUF buffer [64, 128*128] (slot per grid point);
slots stream out to DRAM as they finish.
"""

import numpy as np
import ml_dtypes

import concourse.bacc as bacc
import concourse.bass as bass
import concourse.mybir as mybir
from concourse.tile import TileContext
from concourse.bass_utils import run_bass_kernel_spmd

F32 = mybir.dt.float32
F32R = mybir.dt.float32r
BF16 = mybir.dt.bfloat16
FP16 = mybir.dt.float16
B = 1024
L = 128
C_IN = 8
HID = 64
MLP_H = 128
INIT_H = 20
NSTEP = L - 1  # 127
NCORES = 8
BL = B // NCORES  # 128 batch rows per core

_CACHE: dict = {}


def _flags():
    import os
    return (
        os.environ.get("K_T_F32R", "0") == "1",
        os.environ.get("K_MM2_F32R", "1") == "1",
        os.environ.get("K_MUL_BF16", "1") == "1",
        os.environ.get("K_MM1_F32R", "1") == "1",
        os.environ.get("K_WARM", "0") == "1",
        os.environ.get("K_FP16_PATH", "1") == "1",
        os.environ.get("K_T_FP16", "0") == "1",
        os.environ.get("K_MM1_SPLIT", "1") == "1",
    )


def _build(nstep: int, with_b2: bool):
    import time as _time

    t_f32r, mm2_f32r, mul_bf16, mm1_f32r, warm, fp16_path, t_fp16, mm1_split = _flags()
    TD = F32R if t_f32r else F32
    if t_fp16:
        TD = FP16
    SD = F32R if mm1_f32r else F32
    MD = F32R if mm2_f32r else F32
    UD = BF16 if mul_bf16 else F32
    if fp16_path:
        MD = FP16
        UD = FP16
    t0 = _time.time()
    nc = bacc.Bacc()
    g_in = nc.dram_tensor("g", [BL, nstep * 3 * C_IN], UD, kind="ExternalInput")
    b1_in = nc.dram_tensor("bias1", [MLP_H, nstep * 3], F32, kind="ExternalInput")
    w1z_in = nc.dram_tensor("w1z", [HID, MLP_H], SD, kind="ExternalInput")
    w2_in = nc.dram_tensor("w2", [MLP_H, HID * C_IN], MD, kind="ExternalInput")
    b2_in = nc.dram_tensor("b2r", [1, HID * C_IN], MD, kind="ExternalInput")
    ones_in = nc.dram_tensor("onesr", [1, BL], MD, kind="ExternalInput")
    id_in = nc.dram_tensor("ident", [BL, BL], TD, kind="ExternalInput")
    z0t_in = nc.dram_tensor("z0t", [HID, BL], SD, kind="ExternalInput")
    w1zh_in = nc.dram_tensor("w1zh", [HID, MLP_H], FP16, kind="ExternalInput")
    zs_out = nc.dram_tensor(
        "zs", [HID, (nstep + 1) * BL], F32, kind="ExternalOutput"
    )

    NF = HID * C_IN  # 512
    with TileContext(nc) as tc:
        with (
            tc.tile_pool(name="const", bufs=1) as cp,
            tc.tile_pool(name="zst", bufs=1) as zp,
            tc.tile_pool(name="hs", bufs=3) as hp,
            tc.tile_pool(name="fs", bufs=2) as fp,
            tc.tile_pool(name="us", bufs=2) as up,
            tc.tile_pool(name="ks", bufs=3) as kp,
            tc.tile_pool(name="zc", bufs=3) as zcp,
            tc.tile_pool(name="kh", bufs=2) as khp,
            tc.tile_pool(name="ph", bufs=(4 if mm1_split else 2), space="PSUM") as ph,
            tc.tile_pool(name="pf", bufs=2, space="PSUM") as pf,
            tc.tile_pool(name="pacc", bufs=(1 if mm1_split else 2), space="PSUM") as pacc,
            tc.tile_pool(name="pks", bufs=1, space="PSUM") as pks,
            tc.tile_pool(name="pfill", bufs=1, space="PSUM") as pfill,
        ):
            gS = cp.tile([BL, nstep * 3 * C_IN], UD)
            b1S = cp.tile([MLP_H, nstep * 3], F32)
            w1zS = cp.tile([HID, MLP_H], SD)
            w1zH = cp.tile([HID, MLP_H], FP16)
            w2S = cp.tile([MLP_H, NF], MD)
            b2S = cp.tile([1, NF], MD)
            onesS = cp.tile([1, BL], MD)
            idS = cp.tile([BL, BL], TD)
            zall = zp.tile([HID, (nstep + 1) * BL], SD)
            if warm:
                wt = cp.tile([BL, BL], BF16, name="wt")
                nc.vector.memset(wt[:], 0.0)

            nc.sync.dma_start(out=gS[:], in_=g_in[:])
            nc.sync.dma_start(out=b1S[:], in_=b1_in[:])
            nc.sync.dma_start(out=w1zS[:], in_=w1z_in[:])
            nc.sync.dma_start(out=w1zH[:], in_=w1zh_in[:])
            nc.sync.dma_start(out=w2S[:], in_=w2_in[:])
            nc.sync.dma_start(out=b2S[:], in_=b2_in[:])
            nc.sync.dma_start(out=onesS[:], in_=ones_in[:])
            nc.sync.dma_start(out=idS[:], in_=id_in[:])
            nc.sync.dma_start(out=zall[:, 0:BL], in_=z0t_in[:])
            nc.sync.dma_start(out=zs_out[:, 0:BL], in_=z0t_in[:].bitcast(F32))

            if warm:
                wp = pfill.tile([BL, BL], F32, tag="fl", name="wp")
                for _ in range(48):
                    nc.tensor.matmul(
                        wp[:], lhsT=wt[:], rhs=wt[:], start=True, stop=True
                    )
            CLS = (0, 1, 1, 2)
            ALPHA = (0.5, 0.25, 0.5, 1.0 / 6.0)
            prev_accP = None
            for step in range(nstep):
                zT = zall[:, step * BL : (step + 1) * BL]
                cur = zT
                accP = None
                h_tiles = []
                if mm1_split:
                    zT_prev = zall[:, (step - 1) * BL : step * BL]
                    for s in range(4):
                        h_ps_s = ph.tile([MLP_H, BL], F32, tag="hps", name="hps")
                        has_b = not (step == 0 and s == 0)
                        nc.tensor.matmul(
                            h_ps_s[:],
                            lhsT=w1zS[:],
                            rhs=(zT_prev if (s == 0 and step > 0) else zT),
                            start=True,
                            stop=not has_b,
                        )
                        h_tiles.append(h_ps_s)
                for s in range(4):
                    col = step * 3 + CLS[s]
                    if mm1_split:
                        h_ps = h_tiles[s]
                        has_b = not (step == 0 and s == 0)
                        if has_b:
                            if s == 0:
                                ksrc, alpha_b = prev_accP, 1.0 / 6.0
                            elif s == 1:
                                ksrc, alpha_b = accP, 0.5
                            else:
                                ksrc, alpha_b = prev_ksP, 0.25 if s == 2 else 0.5
                            kh = khp.tile([HID, BL], FP16, tag="kh", name="kh")
                            nc.vector.tensor_scalar_mul(kh[:], ksrc[:], alpha_b)
                            nc.tensor.matmul(
                                h_ps[:],
                                lhsT=w1zH[:],
                                rhs=kh[:],
                                start=False,
                                stop=True,
                            )
                    else:
                        h_ps = ph.tile([MLP_H, BL], F32, tag="hps")
                        nc.tensor.matmul(
                            h_ps[:],
                            lhsT=w1zS[:],
                            rhs=cur,
                            start=True,
                            stop=True,
                        )
                    hS = hp.tile([MLP_H, BL], MD, tag="hs")
                    nc.vector.tensor_scalar(
                        hS[:],
                        h_ps[:],
                        b1S[:, col : col + 1],
                        0.0,
                        op0=mybir.AluOpType.add,
                        op1=mybir.AluOpType.max,
                    )
                    f_ps = pf.tile([BL, NF], F32, tag="fps")
                    if with_b2:
                        nc.tensor.matmul(
                            f_ps[:],
                            lhsT=onesS[:],
                            rhs=b2S[:],
                            start=True,
                            stop=False,
                        )
                    nc.tensor.matmul(
                        f_ps[:],
                        lhsT=hS[:],
                        rhs=w2S[:],
                        start=not with_b2,
                        stop=True,
                    )
                    fS = fp.tile([BL, NF], UD, tag="fs")
                    nc.scalar.activation(
                        fS[:], f_ps[:], mybir.ActivationFunctionType.Tanh
                    )
                    if warm:
                        fl1 = pfill.tile([BL, BL], F32, tag="fl", name="fl1")
                        nc.tensor.matmul(
                            fl1[:],
                            lhsT=fS[:, 0:BL],
                            rhs=fS[:, 0:BL],
                            start=True,
                            stop=True,
                        )
                    u = up.tile([BL, NF], UD, tag="u")
                    f3 = fS[:].rearrange("p (h c) -> p h c", c=C_IN)
                    u3 = u[:].rearrange("p (h c) -> p h c", c=C_IN)
                    gv = (
                        gS[:, col * C_IN : (col + 1) * C_IN]
                        .unsqueeze(1)
                        .broadcast_to((BL, HID, C_IN))
                    )
                    nc.vector.tensor_tensor(
                        out=u3, in0=f3, in1=gv, op=mybir.AluOpType.mult
                    )
                    if warm:
                        fl2 = pfill.tile([BL, BL], F32, tag="fl", name="fl2")
                        nc.tensor.matmul(
                            fl2[:],
                            lhsT=u[:, 0:BL],
                            rhs=u[:, 0:BL],
                            start=True,
                            stop=True,
                        )
                    kn = kp.tile([BL, HID], TD, tag="kn")
                    with nc.allow_low_precision("k reduce output precision"):
                        nc.vector.tensor_reduce(
                            kn[:], u3, axis=mybir.AxisListType.X, op=mybir.AluOpType.add
                        )
                    if s == 0:
                        accP = pacc.tile([HID, BL], TD, tag="acc")
                        nc.tensor.matmul(
                            accP[:],
                            lhsT=kn[:],
                            rhs=idS[:],
                            is_transpose=True,
                            start=True,
                            stop=True,
                        )
                        src = accP
                    elif s in (1, 2):
                        ksP = pks.tile([HID, BL], TD, tag="ks")
                        nc.tensor.matmul(
                            ksP[:],
                            lhsT=kn[:],
                            rhs=idS[:],
                            is_transpose=True,
                            start=True,
                            stop=True,
                        )
                        nc.tensor.matmul(
                            accP[:],
                            lhsT=kn[:],
                            rhs=idS[:],
                            is_transpose=True,
                            start=False,
                            stop=True,
                            skip_group_check=True,
                        )
                        src = ksP
                    else:
                        nc.tensor.matmul(
                            accP[:],
                            lhsT=kn[:],
                            rhs=idS[:],
                            is_transpose=True,
                            start=False,
                            stop=True,
                            skip_group_check=True,
                        )
                        src = accP
                    if s in (1, 2):
                        prev_ksP = ksP
                    if (not mm1_split) or s == 3:
                        if s < 3:
                            out_ap = zcp.tile([HID, BL], SD, tag="zc", name="zc")[:]
                        else:
                            out_ap = zall[:, (step + 1) * BL : (step + 2) * BL]
                        nc.vector.scalar_tensor_tensor(
                            out=out_ap,
                            in0=src[:],
                            scalar=ALPHA[s],
                            in1=zT,
                            op0=mybir.AluOpType.mult,
                            op1=mybir.AluOpType.add,
                        )
                        if s < 3:
                            cur = out_ap
                prev_accP = accP
                nc.sync.dma_start(
                    out=zs_out[:, (step + 1) * BL : (step + 2) * BL],
                    in_=zall[:, (step + 1) * BL : (step + 2) * BL].bitcast(F32),
                )
    import sys

    print(f"[kernel] tile trace+schedule: {_time.time()-t0:.1f}s", file=sys.stderr)
    t1 = _time.time()
    nc.finalize()
    print(f"[kernel] finalize: {_time.time()-t1:.1f}s", file=sys.stderr)
    return nc


def _get_nc(nstep: int, with_b2: bool):
    key = (nstep, with_b2) + _flags()
    if key not in _CACHE:
        _CACHE[key] = _build(nstep, with_b2)
    return _CACHE[key]


def _host_prep(coeffs, Wi1, bi1, Wi2, bi2, W1, b1, W2, b2, nstep: int):
    coeffs = np.asarray(coeffs, dtype=np.float32)
    a = coeffs[:, :, 0:8]
    b = coeffs[:, :, 8:16]
    c = coeffs[:, :, 16:24]
    d = coeffs[:, :, 24:32]

    X0 = a[:, 0]
    z0 = np.tanh(
        np.maximum(X0 @ Wi1 + bi1, 0.0).astype(np.float32) @ Wi2 + bi2
    ).astype(np.float32)

    g = np.empty((B, nstep, 3, C_IN), dtype=np.float32)
    g[:, :, 0] = b[:, :nstep]
    g[:, :, 1] = 2.0 * b[:, :nstep] + 2.0 * c[:, :nstep] + 1.5 * d[:, :nstep]
    # stage-4 derivative: dXdt at t=i+1
    last = NSTEP - 1  # 126 in full problem
    for i in range(nstep):
        if i < last:
            g[:, i, 2] = b[:, i + 1]
        else:
            g[:, i, 2] = b[:, i] + 2.0 * c[:, i] + 3.0 * d[:, i]

    tcols = np.empty((nstep, 3), dtype=np.float32)
    tcols[:, 0] = np.arange(nstep, dtype=np.float32)
    tcols[:, 1] = tcols[:, 0] + 0.5
    tcols[:, 2] = tcols[:, 0] + 1.0
    # bias1[m, step*3+cls] = b1[m] + t * W1[0, m]
    bias1 = (
        b1[None, None, :] + tcols[:, :, None] * W1[0][None, None, :]
    ).astype(np.float32)
    bias1 = bias1.reshape(nstep * 3, MLP_H).T.copy()  # [128, nstep*3]

    wdt = np.float16 if _flags()[5] else np.float32
    shared = {
        "bias1": bias1,
        "w1z": np.ascontiguousarray(W1[1:], dtype=np.float32),
        "w1zh": np.ascontiguousarray(W1[1:], dtype=np.float16),
        "w2": np.ascontiguousarray(W2, dtype=wdt),
        "b2r": np.ascontiguousarray(b2[None, :], dtype=wdt),
        "onesr": np.ones((1, BL), dtype=wdt),
        "ident": np.eye(
            BL, dtype=np.float16 if _flags()[6] else np.float32
        ),
    }
    in_maps = []
    for core in range(NCORES):
        sl = slice(core * BL, (core + 1) * BL)
        m = dict(shared)
        f = _flags()
        gdt = np.float16 if f[5] else (ml_dtypes.bfloat16 if f[2] else np.float32)
        m["g"] = np.ascontiguousarray(
            g[sl].reshape(BL, nstep * 3 * C_IN).astype(gdt)
        )
        m["z0t"] = np.ascontiguousarray(z0[sl].T)
        in_maps.append(m)
    return in_maps, z0


def kernel(coeffs, Wi1, bi1, Wi2, bi2, W1, b1, W2, b2, _nstep: int = NSTEP,
           _trace: bool = False):
    import time as _time
    import sys

    nstep = _nstep
    with_b2 = bool(np.any(np.asarray(b2)))
    nc = _get_nc(nstep, with_b2)
    in_maps, _ = _host_prep(
        coeffs, Wi1, bi1, Wi2, bi2, W1, b1, W2, b2, nstep
    )
    t0 = _time.time()
    res = run_bass_kernel_spmd(nc, in_maps, list(range(NCORES)), trace=_trace)
    print(f"[kernel] spmd run (compile+exec): {_time.time()-t0:.1f}s", file=sys.stderr)
    out = np.empty((B, nstep + 1, HID), dtype=np.float32)
    for core in range(NCORES):
        zs = res.results[core]["zs"].reshape(HID, nstep + 1, BL)
        out[core * BL : (core + 1) * BL] = zs.transpose(2, 1, 0)
    if _trace:
        kernel.last_results = res
    return out



# revision 7
# speedup vs baseline: 1.1027x; 1.1027x over previous
"""NeuralCDE RK4 solver as a Bass/Tile kernel on 8 Trainium2 cores.

Data-parallel over batch: B=1024 -> 128 rows per core. The 127-step RK4
scan is fully unrolled. Transposed pipeline: all tensors keep batch on
the FREE dim so the per-stage recurrence never needs a PE transpose.

State z^T lives padded on 128 partitions: h -> pad(h) = (h//16)*32 + h%16
(16 live + 16 zero rows per 32-block). Per stage:
    stt  (DVE): m = fp16(z^T + alpha * k_prev^T)        [128, B]
    mm1  (PE) : h_psum[128m, B] = w1z_pad.T @ m
    relu (ACT): hS = relu(h_psum + bias1(t))  (time folded in bias)
    mm2T (PE) : fT_psum[128, 4xB] = w2_chunk.T @ hS  (4 chunks of 128
                rows each = (h_local, c) pairs; f comes out TRANSPOSED)
    tanh (ACT): fS = tanh(fT_psum)
    mul  (DVE): uT = fS * gT(step,stage)  (g pre-transposed+replicated)
    red  (PE) : k^T[pad(h), B] accumulated in PSUM via a constant 0/1
                selector stationary S32 -- the c-reduction, the
                transpose, and the RK4 combine all fall out of PSUM
                accumulation for free.
RK4 weights are pre-folded into g (k2,k3 columns hold 2x dXdt).
"""

import numpy as np

import concourse.bacc as bacc
import concourse.bass as bass
import concourse.mybir as mybir
from concourse.tile import TileContext
from concourse.bass_utils import run_bass_kernel_spmd

F32 = mybir.dt.float32
FP16 = mybir.dt.float16
B = 1024
L = 128
C_IN = 8
HID = 64
MLP_H = 128
INIT_H = 20
NSTEP = L - 1  # 127
NCORES = 8
BL = B // NCORES  # 128 batch rows per core
NF = HID * C_IN  # 512
NCH = 4  # f^T chunks of 128 rows (16 h x 8 c each)
HCH = HID // NCH  # 16 live h per chunk

_CACHE: dict = {}


def _flags():
    import os

    return (
        os.environ.get("T_RELU", "act"),      # act | dve
        int(os.environ.get("T_TANH_SPLIT", "2")),
        int(os.environ.get("T_MUL_SPLIT", "2")),
        os.environ.get("T_STT", "dve"),       # dve | pool
        int(os.environ.get("T_GDMA_SLICES", "8")),
    )


def _pad(h):
    return (h // HCH) * (2 * HCH) + (h % HCH)


def _build(nstep: int, with_b2: bool):
    import sys
    import time as _time

    relu_eng, tanh_split, mul_split, stt_eng, gdma_slices = _flags()
    t0 = _time.time()
    nc = bacc.Bacc()
    NCLS = nstep * 3
    gt_in = nc.dram_tensor("gt", [128, NCLS * BL], FP16, kind="ExternalInput")
    b1_in = nc.dram_tensor("bias1", [MLP_H, NCLS], F32, kind="ExternalInput")
    w1z_in = nc.dram_tensor("w1z", [HID, MLP_H], FP16, kind="ExternalInput")
    w2_in = nc.dram_tensor("w2", [MLP_H, NF], FP16, kind="ExternalInput")
    s64_in = nc.dram_tensor("s64", [128, NCH * HID], FP16, kind="ExternalInput")
    b2p_in = nc.dram_tensor("b2p", [1, NF], F32, kind="ExternalInput")
    onesr_in = nc.dram_tensor("onesr", [1, BL], F32, kind="ExternalInput")
    z0t_in = nc.dram_tensor("z0t", [HID, BL], F32, kind="ExternalInput")
    m0_in = nc.dram_tensor("m0", [HID, BL], FP16, kind="ExternalInput")
    zs_out = nc.dram_tensor(
        "zs", [HID, (nstep + 1) * BL], F32, kind="ExternalOutput"
    )

    CLS = (0, 1, 1, 2)

    with TileContext(nc) as tc:
        with (
            tc.tile_pool(name="const", bufs=1) as cp,
            tc.tile_pool(name="zst", bufs=1) as zp,
            tc.tile_pool(name="ms", bufs=3) as mp,
            tc.tile_pool(name="hs", bufs=3) as hp,
            tc.tile_pool(name="fs", bufs=2) as fp,
            tc.tile_pool(name="us", bufs=2) as up,
            tc.tile_pool(name="ph", bufs=2, space="PSUM") as ph,
            tc.tile_pool(name="pf", bufs=2, space="PSUM") as pf,
            tc.tile_pool(name="pacc", bufs=2, space="PSUM") as pacc,
            tc.tile_pool(name="pks", bufs=2, space="PSUM") as pks,
        ):
            gtS = cp.tile([128, NCLS * BL], FP16)
            b1S = cp.tile([MLP_H, NCLS], F32)
            w1zS = cp.tile([HID, MLP_H], FP16)
            w2S = cp.tile([MLP_H, NF], FP16)
            s64S = cp.tile([128, NCH * HID], FP16)
            b2S = cp.tile([1, NF], F32)
            onesS = cp.tile([1, BL], F32)
            m0S = cp.tile([HID, BL], FP16)
            zall = zp.tile([HID, (nstep + 1) * BL], F32)

            # gt is big (~12 MB): slice the load so step 0 isn't gated on
            # the whole transfer.
            nsl = gdma_slices
            per = (NCLS + nsl - 1) // nsl
            for i in range(nsl):
                lo = i * per * BL
                hi = min(NCLS * BL, (i + 1) * per * BL)
                if lo >= hi:
                    break
                nc.sync.dma_start(out=gtS[:, lo:hi], in_=gt_in[:, lo:hi])
            nc.sync.dma_start(out=b1S[:], in_=b1_in[:])
            nc.sync.dma_start(out=w1zS[:], in_=w1z_in[:])
            nc.sync.dma_start(out=w2S[:], in_=w2_in[:])
            nc.sync.dma_start(out=s64S[:], in_=s64_in[:])
            nc.sync.dma_start(out=b2S[:], in_=b2p_in[:])
            nc.sync.dma_start(out=onesS[:], in_=onesr_in[:])
            nc.sync.dma_start(out=m0S[:], in_=m0_in[:])
            nc.sync.dma_start(out=zall[:, 0:BL], in_=z0t_in[:])
            nc.sync.dma_start(out=zs_out[:, 0:BL], in_=z0t_in[:])

            stt = nc.vector.scalar_tensor_tensor
            accP = None
            prev_ksP = None
            for step in range(nstep):
                zT = zall[:, step * BL : (step + 1) * BL]
                for s in range(4):
                    col = step * 3 + CLS[s]
                    # ---- m (fp16 moving operand for mm1) ----
                    if step == 0 and s == 0:
                        m = m0S
                    else:
                        if s == 0:
                            ksrc, al = accP, 1.0 / 6.0
                            # z update: z_step = z_{step-1} + accP/6
                            stt(
                                out=zT,
                                in0=accP[:],
                                scalar=al,
                                in1=zall[:, (step - 1) * BL : step * BL],
                                op0=mybir.AluOpType.mult,
                                op1=mybir.AluOpType.add,
                            )
                            nc.sync.dma_start(
                                out=zs_out[:, step * BL : (step + 1) * BL],
                                in_=zT,
                            )
                            zbase = zall[:, (step - 1) * BL : step * BL]
                        elif s == 1:
                            ksrc, al = accP, 0.5
                            zbase = zT
                        else:
                            ksrc, al = prev_ksP, (0.25 if s == 2 else 0.5)
                            zbase = zT
                        m = mp.tile([HID, BL], FP16, tag="m")
                        stt(
                            out=m[:],
                            in0=ksrc[:],
                            scalar=al,
                            in1=zbase,
                            op0=mybir.AluOpType.mult,
                            op1=mybir.AluOpType.add,
                        )
                    # ---- mm1 ----
                    h_ps = ph.tile([MLP_H, BL], F32, tag="hps")
                    nc.tensor.matmul(
                        h_ps[:], lhsT=w1zS[:], rhs=m[:], start=True, stop=True
                    )
                    # ---- relu (+bias with time folded in) ----
                    hS = hp.tile([MLP_H, BL], FP16, tag="hs")
                    if relu_eng == "act":
                        nc.scalar.activation(
                            hS[:],
                            h_ps[:],
                            mybir.ActivationFunctionType.Relu,
                            bias=b1S[:, col : col + 1],
                        )
                    else:
                        nc.vector.tensor_scalar(
                            hS[:],
                            h_ps[:],
                            b1S[:, col : col + 1],
                            0.0,
                            op0=mybir.AluOpType.add,
                            op1=mybir.AluOpType.max,
                        )
                    # ---- mm2 transposed (4 chunks) + tanh + mul + red ----
                    fT = pf.tile([128, NCH * BL], F32, tag="fps")
                    fS = fp.tile([128, NCH * BL], FP16, tag="fs")
                    u = up.tile([128, NCH * BL], FP16, tag="u")
                    if s == 0:
                        accP = pacc.tile([HID, BL], F32, tag="acc")
                    if s in (1, 2):
                        ksP = pks.tile([HID, BL], F32, tag="ks")
                        kdst = ksP
                    else:
                        kdst = accP
                    per_t = NCH // tanh_split
                    per_m = NCH // mul_split
                    done_t = 0
                    done_m = 0
                    for cch in range(NCH):
                        csl = slice(cch * BL, (cch + 1) * BL)
                        if with_b2:
                            nc.tensor.matmul(
                                fT[:, csl],
                                lhsT=b2S[:, cch * MLP_H : (cch + 1) * MLP_H],
                                rhs=onesS[:],
                                start=True,
                                stop=False,
                            )
                        nc.tensor.matmul(
                            fT[:, csl],
                            lhsT=w2S[:, cch * MLP_H : (cch + 1) * MLP_H],
                            rhs=hS[:],
                            start=not with_b2,
                            stop=True,
                        )
                        if cch + 1 - done_t >= per_t:
                            tsl = slice(done_t * BL, (cch + 1) * BL)
                            nc.scalar.activation(
                                fS[:, tsl],
                                fT[:, tsl],
                                mybir.ActivationFunctionType.Tanh,
                            )
                            done_t = cch + 1
                        if cch + 1 - done_m >= per_m and done_t == cch + 1:
                            n = cch + 1 - done_m
                            f3 = fS[:, done_m * BL : (cch + 1) * BL].rearrange(
                                "p (ch b) -> p ch b", ch=n
                            )
                            u3 = u[:, done_m * BL : (cch + 1) * BL].rearrange(
                                "p (ch b) -> p ch b", ch=n
                            )
                            gvn = (
                                gtS[:, col * BL : (col + 1) * BL]
                                .unsqueeze(1)
                                .broadcast_to((128, n, BL))
                            )
                            nc.vector.tensor_tensor(
                                out=u3, in0=f3, in1=gvn, op=mybir.AluOpType.mult
                            )
                            for rch in range(done_m, cch + 1):
                                rsl = slice(rch * BL, (rch + 1) * BL)
                                ssl = slice(rch * HID, (rch + 1) * HID)
                                fresh = (s != 3) and rch == 0
                                nc.tensor.matmul(
                                    kdst[:],
                                    lhsT=s64S[:, ssl],
                                    rhs=u[:, rsl],
                                    start=fresh,
                                    stop=(rch == NCH - 1),
                                    skip_group_check=True,
                                )
                                if s in (1, 2):
                                    nc.tensor.matmul(
                                        accP[:],
                                        lhsT=s64S[:, ssl],
                                        rhs=u[:, rsl],
                                        start=False,
                                        stop=(rch == NCH - 1),
                                        skip_group_check=True,
                                    )
                            done_m = cch + 1
                    if s in (1, 2):
                        prev_ksP = ksP
            # epilogue: final z
            zT = zall[:, nstep * BL : (nstep + 1) * BL]
            stt(
                out=zT,
                in0=accP[:],
                scalar=1.0 / 6.0,
                in1=zall[:, (nstep - 1) * BL : nstep * BL],
                op0=mybir.AluOpType.mult,
                op1=mybir.AluOpType.add,
            )
            nc.sync.dma_start(
                out=zs_out[:, nstep * BL : (nstep + 1) * BL], in_=zT
            )

    print(f"[kernel] tile trace+schedule: {_time.time()-t0:.1f}s", file=sys.stderr)
    t1 = _time.time()
    nc.finalize()
    print(f"[kernel] finalize: {_time.time()-t1:.1f}s", file=sys.stderr)
    return nc


def _get_nc(nstep: int, with_b2: bool):
    key = (nstep, with_b2) + _flags()
    if key not in _CACHE:
        _CACHE[key] = _build(nstep, with_b2)
    return _CACHE[key]


def _host_prep(coeffs, Wi1, bi1, Wi2, bi2, W1, b1, W2, b2, nstep: int):
    coeffs = np.asarray(coeffs, dtype=np.float32)
    a = coeffs[:, :, 0:8]
    b = coeffs[:, :, 8:16]
    c = coeffs[:, :, 16:24]
    d = coeffs[:, :, 24:32]

    X0 = a[:, 0]
    z0 = np.tanh(
        np.maximum(X0 @ Wi1 + bi1, 0.0).astype(np.float32) @ Wi2 + bi2
    ).astype(np.float32)

    # g with RK4 weights folded (cls1 column = 2x dXdt(t+1/2))
    g = np.empty((B, nstep, 3, C_IN), dtype=np.float32)
    g[:, :, 0] = b[:, :nstep]
    g[:, :, 1] = 2.0 * b[:, :nstep] + 2.0 * c[:, :nstep] + 1.5 * d[:, :nstep]
    last = NSTEP - 1
    for i in range(nstep):
        if i < last:
            g[:, i, 2] = b[:, i + 1]
        else:
            g[:, i, 2] = b[:, i] + 2.0 * c[:, i] + 3.0 * d[:, i]

    tcols = np.empty((nstep, 3), dtype=np.float32)
    tcols[:, 0] = np.arange(nstep, dtype=np.float32)
    tcols[:, 1] = tcols[:, 0] + 0.5
    tcols[:, 2] = tcols[:, 0] + 1.0
    bias1 = (
        b1[None, None, :] + tcols[:, :, None] * W1[0][None, None, :]
    ).astype(np.float32)
    bias1 = bias1.reshape(nstep * 3, MLP_H).T.copy()  # [128, nstep*3]

    # per-chunk selectors: s64[(hl*8+c), chunk*64 + h'] = 1 if h' == chunk*16+hl
    s64 = np.zeros((128, NCH * HID), dtype=np.float16)
    rows = np.arange(128)
    for cch in range(NCH):
        s64[rows, cch * HID + cch * HCH + rows // C_IN] = 1.0

    shared = {
        "bias1": bias1,
        "w1z": np.ascontiguousarray(W1[1:], dtype=np.float16),
        "w2": np.ascontiguousarray(W2, dtype=np.float16),
        "s64": s64,
        "b2p": np.ascontiguousarray(b2[None, :], dtype=np.float32),
        "onesr": np.ones((1, BL), dtype=np.float32),
    }
    in_maps = []
    for core in range(NCORES):
        sl = slice(core * BL, (core + 1) * BL)
        mm = dict(shared)
        # gt[r, step, cls, b] = g[b, step, cls, r % 8], replicated 16x
        gcore = g[sl].transpose(3, 1, 2, 0)  # [8, nstep, 3, BL]
        gt = np.tile(gcore, (HCH, 1, 1, 1)).reshape(128, nstep * 3 * BL)
        mm["gt"] = np.ascontiguousarray(gt.astype(np.float16))
        z0t = np.ascontiguousarray(z0[sl].T)
        mm["z0t"] = z0t
        mm["m0"] = z0t.astype(np.float16)
        in_maps.append(mm)
    return in_maps


def kernel(coeffs, Wi1, bi1, Wi2, bi2, W1, b1, W2, b2, _nstep: int = NSTEP,
           _trace: bool = False):
    import sys
    import time as _time

    nstep = _nstep
    with_b2 = bool(np.any(np.asarray(b2)))
    nc = _get_nc(nstep, with_b2)
    in_maps = _host_prep(
        coeffs, Wi1, bi1, Wi2, bi2, W1, b1, W2, b2, nstep
    )
    t0 = _time.time()
    res = run_bass_kernel_spmd(nc, in_maps, list(range(NCORES)), trace=_trace)
    print(f"[kernel] spmd run (compile+exec): {_time.time()-t0:.1f}s", file=sys.stderr)
    out = np.empty((B, nstep + 1, HID), dtype=np.float32)
    for core in range(NCORES):
        zs = res.results[core]["zs"].reshape(HID, nstep + 1, BL)
        out[core * BL : (core + 1) * BL] = zs.transpose(2, 1, 0)
    if _trace:
        kernel.last_results = res
    return out


# revision 10
# speedup vs baseline: 1.3061x; 1.1845x over previous
"""NeuralCDE RK4 solver as a Bass/Tile kernel on 8 Trainium2 cores.

Data-parallel over batch: B=1024 -> 128 rows per core. The 127-step RK4
scan is fully unrolled. Transposed pipeline: all tensors keep batch on
the FREE dim so the per-stage recurrence never needs a PE transpose.

State z^T lives padded on 128 partitions: h -> pad(h) = (h//16)*32 + h%16
(16 live + 16 zero rows per 32-block). Per stage:
    stt  (DVE): m = fp16(z^T + alpha * k_prev^T)        [128, B]
    mm1  (PE) : h_psum[128m, B] = w1z_pad.T @ m
    relu (ACT): hS = relu(h_psum + bias1(t))  (time folded in bias)
    mm2T (PE) : fT_psum[128, 4xB] = w2_chunk.T @ hS  (4 chunks of 128
                rows each = (h_local, c) pairs; f comes out TRANSPOSED)
    tanh (ACT): fS = tanh(fT_psum)
    mul  (DVE): uT = fS * gT(step,stage)  (g pre-transposed+replicated)
    red  (PE) : k^T[pad(h), B] accumulated in PSUM via a constant 0/1
                selector stationary S32 -- the c-reduction, the
                transpose, and the RK4 combine all fall out of PSUM
                accumulation for free.
RK4 weights are pre-folded into g (k2,k3 columns hold 2x dXdt).
"""

import numpy as np

import concourse.bacc as bacc
import concourse.bass as bass
import concourse.mybir as mybir
from concourse.tile import TileContext
from concourse.bass_utils import run_bass_kernel_spmd

F32 = mybir.dt.float32
FP16 = mybir.dt.float16
B = 1024
L = 128
C_IN = 8
HID = 64
MLP_H = 128
INIT_H = 20
NSTEP = L - 1  # 127
NCORES = 8
BL = B // NCORES  # 128 batch rows per core
NF = HID * C_IN  # 512
NCH = 4  # f^T chunks of 128 rows (16 h x 8 c each)
HCH = HID // NCH  # 16 live h per chunk

_CACHE: dict = {}


def _flags():
    import os

    return (
        os.environ.get("T_RELU", "act"),      # act | dve
        int(os.environ.get("T_TANH_SPLIT", "2")),
        int(os.environ.get("T_MUL_SPLIT", "2")),
        os.environ.get("T_STT", "dve"),       # dve | pool
        int(os.environ.get("T_GDMA_SLICES", "8")),
    )


def _pad(h):
    return (h // HCH) * (2 * HCH) + (h % HCH)


def _build(nstep: int, with_b2: bool):
    import sys
    import time as _time

    relu_eng, tanh_split, mul_split, stt_eng, gdma_slices = _flags()
    t0 = _time.time()
    nc = bacc.Bacc()
    NCLS = nstep * 3
    gt_in = nc.dram_tensor("gt", [128, NCLS * BL], FP16, kind="ExternalInput")
    b1_in = nc.dram_tensor("bias1", [MLP_H, NCLS], F32, kind="ExternalInput")
    w1z_in = nc.dram_tensor("w1z", [HID, MLP_H], FP16, kind="ExternalInput")
    w2_in = nc.dram_tensor("w2", [MLP_H, NF], FP16, kind="ExternalInput")
    s64_in = nc.dram_tensor("s64", [128, NCH * HID], FP16, kind="ExternalInput")
    b2p_in = nc.dram_tensor("b2p", [1, NF], F32, kind="ExternalInput")
    onesr_in = nc.dram_tensor("onesr", [1, BL], F32, kind="ExternalInput")
    z0t_in = nc.dram_tensor("z0t", [HID, BL], F32, kind="ExternalInput")
    m0_in = nc.dram_tensor("m0", [HID, BL], FP16, kind="ExternalInput")
    zs_out = nc.dram_tensor(
        "zs", [HID, (nstep + 1) * BL], F32, kind="ExternalOutput"
    )

    CLS = (0, 1, 1, 2)

    with TileContext(nc) as tc:
        with (
            tc.tile_pool(name="const", bufs=1) as cp,
            tc.tile_pool(name="zst", bufs=1) as zp,
            tc.tile_pool(name="ms", bufs=3) as mp,
            tc.tile_pool(name="hs", bufs=3) as hp,
            tc.tile_pool(name="fs", bufs=2) as fp,
            tc.tile_pool(name="us", bufs=3) as up,
            tc.tile_pool(name="ph", bufs=2, space="PSUM") as ph,
            tc.tile_pool(name="pf", bufs=1, space="PSUM") as pf,
            tc.tile_pool(name="pacc", bufs=1, space="PSUM") as pacc,
            tc.tile_pool(name="pks", bufs=2, space="PSUM") as pks,
        ):
            gtS = cp.tile([128, NCLS * BL], FP16)
            b1S = cp.tile([MLP_H, NCLS], F32)
            w1zS = cp.tile([HID, MLP_H], FP16)
            w2S = cp.tile([MLP_H, NF], FP16)
            s64S = cp.tile([128, NCH * HID], FP16)
            b2S = cp.tile([1, NF], F32)
            onesS = cp.tile([1, BL], F32)
            m0S = cp.tile([HID, BL], FP16)
            zall = zp.tile([HID, (nstep + 1) * BL], F32)

            # gt is big (~12 MB): slice the load so step 0 isn't gated on
            # the whole transfer.
            nsl = gdma_slices
            per = (NCLS + nsl - 1) // nsl
            for i in range(nsl):
                lo = i * per * BL
                hi = min(NCLS * BL, (i + 1) * per * BL)
                if lo >= hi:
                    break
                nc.sync.dma_start(out=gtS[:, lo:hi], in_=gt_in[:, lo:hi])
            nc.sync.dma_start(out=b1S[:], in_=b1_in[:])
            nc.sync.dma_start(out=w1zS[:], in_=w1z_in[:])
            nc.sync.dma_start(out=w2S[:], in_=w2_in[:])
            nc.sync.dma_start(out=s64S[:], in_=s64_in[:])
            nc.sync.dma_start(out=b2S[:], in_=b2p_in[:])
            nc.sync.dma_start(out=onesS[:], in_=onesr_in[:])
            nc.sync.dma_start(out=m0S[:], in_=m0_in[:])
            nc.sync.dma_start(out=zall[:, 0:BL], in_=z0t_in[:])
            nc.sync.dma_start(out=zs_out[:, 0:BL], in_=z0t_in[:])

            stt = nc.vector.scalar_tensor_tensor
            accP = None
            prev_ksP = None
            pending_acc = []
            for step in range(nstep):
                zT = zall[:, step * BL : (step + 1) * BL]
                for s in range(4):
                    col = step * 3 + CLS[s]
                    # ---- m (fp16 moving operand for mm1) ----
                    if step == 0 and s == 0:
                        m = m0S
                    else:
                        if s == 0:
                            ksrc, al = accP, 1.0 / 6.0
                            # z update: z_step = z_{step-1} + accP/6
                            stt(
                                out=zT,
                                in0=accP[:],
                                scalar=al,
                                in1=zall[:, (step - 1) * BL : step * BL],
                                op0=mybir.AluOpType.mult,
                                op1=mybir.AluOpType.add,
                            )
                            nc.sync.dma_start(
                                out=zs_out[:, step * BL : (step + 1) * BL],
                                in_=zT,
                            )
                            zbase = zall[:, (step - 1) * BL : step * BL]
                        elif s == 1:
                            ksrc, al = accP, 0.5
                            zbase = zT
                        else:
                            ksrc, al = prev_ksP, (0.25 if s == 2 else 0.5)
                            zbase = zT
                        m = mp.tile([HID, BL], FP16, tag="m")
                        stt(
                            out=m[:],
                            in0=ksrc[:],
                            scalar=al,
                            in1=zbase,
                            op0=mybir.AluOpType.mult,
                            op1=mybir.AluOpType.add,
                        )
                    # ---- mm1 ----
                    h_ps = ph.tile([MLP_H, BL], F32, tag="hps")
                    nc.tensor.matmul(
                        h_ps[:], lhsT=w1zS[:], rhs=m[:], start=True, stop=True
                    )
                    # flush deferred accP accumulation into the PE stream
                    # here (after mm1, before this stage's mm2T chunks)
                    while pending_acc:
                        puh, paccP = pending_acc.pop(0)
                        for cch in range(NCH):
                            pu = puh[cch // (NCH // 2)]
                            rsl = slice((cch % (NCH // 2)) * BL,
                                        (cch % (NCH // 2) + 1) * BL)
                            ssl = slice(cch * HID, (cch + 1) * HID)
                            nc.tensor.matmul(
                                paccP[:],
                                lhsT=s64S[:, ssl],
                                rhs=pu[:, rsl],
                                start=False,
                                stop=True,
                                skip_group_check=True,
                            )
                    # ---- relu (+bias with time folded in) ----
                    hS = hp.tile([MLP_H, BL], FP16, tag="hs")
                    if relu_eng == "act":
                        nc.scalar.activation(
                            hS[:],
                            h_ps[:],
                            mybir.ActivationFunctionType.Relu,
                            bias=b1S[:, col : col + 1],
                        )
                    else:
                        nc.vector.tensor_scalar(
                            hS[:],
                            h_ps[:],
                            b1S[:, col : col + 1],
                            0.0,
                            op0=mybir.AluOpType.add,
                            op1=mybir.AluOpType.max,
                        )
                    # ---- mm2 transposed (4 chunks) + tanh + mul + red ----
                    # per-half tiles: a tanh read of half 0 must not create a
                    # WAR hazard against mm2T writes of half 1
                    NH = 2
                    CPH = NCH // NH  # chunks per half
                    fTh = [pf.tile([128, CPH * BL], F32, tag=f"fps{hh}",
                                   name=f"fT{hh}") for hh in range(NH)]
                    fSh = [fp.tile([128, CPH * BL], FP16, tag=f"fs{hh}",
                                   name=f"fS{hh}") for hh in range(NH)]
                    uh = [up.tile([128, CPH * BL], FP16, tag=f"u{hh}",
                                  name=f"u{hh}") for hh in range(NH)]
                    if s == 0:
                        accP = pacc.tile([HID, BL], F32, tag="acc")
                    if s in (1, 2):
                        ksP = pks.tile([HID, BL], F32, tag="ks")
                        kdst = ksP
                    else:
                        kdst = accP
                    gcol = gtS[:, col * BL : (col + 1) * BL]
                    for hh in range(NH):
                        fT, fS, u = fTh[hh], fSh[hh], uh[hh]
                        for lc in range(CPH):
                            cch = hh * CPH + lc
                            csl = slice(lc * BL, (lc + 1) * BL)
                            if with_b2:
                                nc.tensor.matmul(
                                    fT[:, csl],
                                    lhsT=b2S[:, cch * MLP_H : (cch + 1) * MLP_H],
                                    rhs=onesS[:],
                                    start=True,
                                    stop=False,
                                )
                            nc.tensor.matmul(
                                fT[:, csl],
                                lhsT=w2S[:, cch * MLP_H : (cch + 1) * MLP_H],
                                rhs=hS[:],
                                start=not with_b2,
                                stop=True,
                            )
                        nc.scalar.activation(
                            fS[:], fT[:], mybir.ActivationFunctionType.Tanh
                        )
                        f3 = fS[:].rearrange("p (ch b) -> p ch b", ch=CPH)
                        u3 = u[:].rearrange("p (ch b) -> p ch b", ch=CPH)
                        gvn = gcol.unsqueeze(1).broadcast_to((128, CPH, BL))
                        nc.vector.tensor_tensor(
                            out=u3, in0=f3, in1=gvn, op=mybir.AluOpType.mult
                        )
                        for lc in range(CPH):
                            cch = hh * CPH + lc
                            rsl = slice(lc * BL, (lc + 1) * BL)
                            ssl = slice(cch * HID, (cch + 1) * HID)
                            fresh = (s != 3) and cch == 0
                            nc.tensor.matmul(
                                kdst[:],
                                lhsT=s64S[:, ssl],
                                rhs=u[:, rsl],
                                start=fresh,
                                stop=(cch == NCH - 1),
                                skip_group_check=True,
                            )
                    if s in (1, 2):
                        # defer the accP duplicates: they are off the critical
                        # chain, so run them one stage later in PE idle time
                        pending_acc.append((uh, accP))
                        prev_ksP = ksP
            # epilogue: flush remaining accP dups, then final z
            while pending_acc:
                puh, paccP = pending_acc.pop(0)
                for cch in range(NCH):
                    pu = puh[cch // (NCH // 2)]
                    rsl = slice((cch % (NCH // 2)) * BL,
                                (cch % (NCH // 2) + 1) * BL)
                    ssl = slice(cch * HID, (cch + 1) * HID)
                    nc.tensor.matmul(
                        paccP[:],
                        lhsT=s64S[:, ssl],
                        rhs=pu[:, rsl],
                        start=False,
                        stop=True,
                        skip_group_check=True,
                    )
            zT = zall[:, nstep * BL : (nstep + 1) * BL]
            stt(
                out=zT,
                in0=accP[:],
                scalar=1.0 / 6.0,
                in1=zall[:, (nstep - 1) * BL : nstep * BL],
                op0=mybir.AluOpType.mult,
                op1=mybir.AluOpType.add,
            )
            nc.sync.dma_start(
                out=zs_out[:, nstep * BL : (nstep + 1) * BL], in_=zT
            )

    print(f"[kernel] tile trace+schedule: {_time.time()-t0:.1f}s", file=sys.stderr)
    t1 = _time.time()
    nc.finalize()
    print(f"[kernel] finalize: {_time.time()-t1:.1f}s", file=sys.stderr)
    return nc


def _get_nc(nstep: int, with_b2: bool):
    key = (nstep, with_b2) + _flags()
    if key not in _CACHE:
        _CACHE[key] = _build(nstep, with_b2)
    return _CACHE[key]


def _host_prep(coeffs, Wi1, bi1, Wi2, bi2, W1, b1, W2, b2, nstep: int):
    coeffs = np.asarray(coeffs, dtype=np.float32)
    a = coeffs[:, :, 0:8]
    b = coeffs[:, :, 8:16]
    c = coeffs[:, :, 16:24]
    d = coeffs[:, :, 24:32]

    X0 = a[:, 0]
    z0 = np.tanh(
        np.maximum(X0 @ Wi1 + bi1, 0.0).astype(np.float32) @ Wi2 + bi2
    ).astype(np.float32)

    # g with RK4 weights folded (cls1 column = 2x dXdt(t+1/2))
    g = np.empty((B, nstep, 3, C_IN), dtype=np.float32)
    g[:, :, 0] = b[:, :nstep]
    g[:, :, 1] = 2.0 * b[:, :nstep] + 2.0 * c[:, :nstep] + 1.5 * d[:, :nstep]
    last = NSTEP - 1
    for i in range(nstep):
        if i < last:
            g[:, i, 2] = b[:, i + 1]
        else:
            g[:, i, 2] = b[:, i] + 2.0 * c[:, i] + 3.0 * d[:, i]

    tcols = np.empty((nstep, 3), dtype=np.float32)
    tcols[:, 0] = np.arange(nstep, dtype=np.float32)
    tcols[:, 1] = tcols[:, 0] + 0.5
    tcols[:, 2] = tcols[:, 0] + 1.0
    bias1 = (
        b1[None, None, :] + tcols[:, :, None] * W1[0][None, None, :]
    ).astype(np.float32)
    bias1 = bias1.reshape(nstep * 3, MLP_H).T.copy()  # [128, nstep*3]

    # per-chunk selectors: s64[(hl*8+c), chunk*64 + h'] = 1 if h' == chunk*16+hl
    s64 = np.zeros((128, NCH * HID), dtype=np.float16)
    rows = np.arange(128)
    for cch in range(NCH):
        s64[rows, cch * HID + cch * HCH + rows // C_IN] = 1.0

    shared = {
        "bias1": bias1,
        "w1z": np.ascontiguousarray(W1[1:], dtype=np.float16),
        "w2": np.ascontiguousarray(W2, dtype=np.float16),
        "s64": s64,
        "b2p": np.ascontiguousarray(b2[None, :], dtype=np.float32),
        "onesr": np.ones((1, BL), dtype=np.float32),
    }
    in_maps = []
    for core in range(NCORES):
        sl = slice(core * BL, (core + 1) * BL)
        mm = dict(shared)
        # gt[r, step, cls, b] = g[b, step, cls, r % 8], replicated 16x
        gcore = g[sl].transpose(3, 1, 2, 0)  # [8, nstep, 3, BL]
        gt = np.tile(gcore, (HCH, 1, 1, 1)).reshape(128, nstep * 3 * BL)
        mm["gt"] = np.ascontiguousarray(gt.astype(np.float16))
        z0t = np.ascontiguousarray(z0[sl].T)
        mm["z0t"] = z0t
        mm["m0"] = z0t.astype(np.float16)
        in_maps.append(mm)
    return in_maps


def kernel(coeffs, Wi1, bi1, Wi2, bi2, W1, b1, W2, b2, _nstep: int = NSTEP,
           _trace: bool = False):
    import sys
    import time as _time

    nstep = _nstep
    with_b2 = bool(np.any(np.asarray(b2)))
    nc = _get_nc(nstep, with_b2)
    in_maps = _host_prep(
        coeffs, Wi1, bi1, Wi2, bi2, W1, b1, W2, b2, nstep
    )
    t0 = _time.time()
    res = run_bass_kernel_spmd(nc, in_maps, list(range(NCORES)), trace=_trace)
    print(f"[kernel] spmd run (compile+exec): {_time.time()-t0:.1f}s", file=sys.stderr)
    out = np.empty((B, nstep + 1, HID), dtype=np.float32)
    for core in range(NCORES):
        zs = res.results[core]["zs"].reshape(HID, nstep + 1, BL)
        out[core * BL : (core + 1) * BL] = zs.transpose(2, 1, 0)
    if _trace:
        kernel.last_results = res
    return out
